# Initial kernel scaffold
#
"""Trainium2 Bass kernel for the ActorMCP mixture-of-experts policy network.

Data-parallel over 8 NeuronCores: batch 32768 -> 4096 rows/core, weights
replicated. All activations live transposed [feature(partitions), batch(free)]
so every layer is out.T = W.T @ x.T with W in its natural [fan_in, fan_out]
layout as the stationary matmul operand.
"""

import math

import ml_dtypes
import numpy as np

import concourse.mybir as mybir
import concourse.tile as tile
from concourse import bacc, bass_utils

# Problem shape constants (fixed by the task).
B = 32768
IN_DIM = 512
IN_DIM_NG = 480
H1, H2 = 512, 256
E, EH = 8, 256
A = 12                    # ACT_DIM
G = E * A                 # 96 rows: (expert, action) pairs
NCORES = 8
BL = B // NCORES          # 4096 batch rows per core
NB = 512                  # batch columns per tile
NT = BL // NB             # batch tiles per core
OUT_ROWS = 2 * A + E      # mu(12) + sigma(12) + weights(8)

F32 = mybir.dt.float32
BF16 = mybir.dt.bfloat16
AF = mybir.ActivationFunctionType
OP = mybir.AluOpType

IVAR_LO = math.exp(-4.0)  # clip(log_std, -5, 2) in exp(-2*ls) domain
IVAR_HI = math.exp(10.0)

_CACHE: dict = {}


def _build_nc():
    nc = bacc.Bacc(
        "TRN2", target_bir_lowering=False, debug=False, num_devices=NCORES
    )

    def din(name, shape, dt):
        return nc.dram_tensor(name, list(shape), dt, kind="ExternalInput").ap()

    xT = din("xT", (IN_DIM, BL), BF16)
    xngT = din("xngT", (IN_DIM_NG, BL), BF16)
    gW1 = din("gW1", (IN_DIM, H1), BF16)
    gb1 = din("gb1", (H1, 1), F32)
    gW2 = din("gW2", (H1, H2), BF16)
    gb2 = din("gb2", (H2, 1), F32)
    gW3x = din("gW3x", (H2, G), BF16)     # gW3 columns repeated 12x
    gb3x = din("gb3x", (G, 1), F32)
    eW1 = din("eW1", (IN_DIM_NG, H1), BF16)
    eb1 = din("eb1", (H1, 1), F32)
    eW2 = din("eW2", (H1, H2), BF16)
    eb2 = din("eb2", (H2, 1), F32)
    hW = din("hW", (H2, E * EH), BF16)
    hb = din("hb", (E * EH, 1), F32)
    Wmu = din("Wmu", (E * EH, G), BF16)   # block-diagonal per-expert mu GEMM
    Wls = din("Wls", (E * EH, G), BF16)   # block-diagonal per-expert log_std
    cbmu = din("cbmu", (G, 1), F32)
    cbls2 = din("cbls2", (G, 1), F32)     # -2 * conv bias (log_std half)
    Ssum = din("Ssum", (G, A), BF16)      # sums (e, a) rows over e

    out = nc.dram_tensor("out", [OUT_ROWS, BL], F32, kind="ExternalOutput").ap()

    with tile.TileContext(nc) as tc:
        with (
            tc.tile_pool(name="wpool", bufs=1) as wpool,
            tc.tile_pool(name="apool", bufs=2) as apool,
            tc.tile_pool(name="psum", bufs=1, space="PSUM") as psum,
        ):
            def load_w(ap, K, name):
                tiles = []
                for k0 in range(0, K, 128):
                    kk = min(128, K - k0)
                    t = wpool.tile(
                        [kk, ap.shape[1]], BF16, name=f"{name}k{k0}",
                        tag=f"{name}k{k0}", bufs=1,
                    )
                    nc.sync.dma_start(t[:], ap[k0:k0 + kk, :])
                    tiles.append(t)
                return tiles

            def load_b(ap, K, name):
                tiles = []
                for k0 in range(0, K, 128):
                    kk = min(128, K - k0)
                    t = wpool.tile(
                        [kk, 1], F32, name=f"{name}k{k0}",
                        tag=f"{name}k{k0}", bufs=1,
                    )
                    nc.sync.dma_start(t[:], ap[k0:k0 + kk, :])
                    tiles.append(t)
                return tiles

            gW1_t = load_w(gW1, IN_DIM, "gW1")
            gW2_t = load_w(gW2, H1, "gW2")
            gW3_t = load_w(gW3x, H2, "gW3")
            eW1_t = load_w(eW1, IN_DIM_NG, "eW1")
            eW2_t = load_w(eW2, H1, "eW2")
            hW_t = load_w(hW, H2, "hW")
            Wmu_t = load_w(Wmu, E * EH, "Wmu")
            Wls_t = load_w(Wls, E * EH, "Wls")
            gb1_t = load_b(gb1, H1, "gb1")
            gb2_t = load_b(gb2, H2, "gb2")
            gb3_t = load_b(gb3x, G, "gb3")
            eb1_t = load_b(eb1, H1, "eb1")
            eb2_t = load_b(eb2, H2, "eb2")
            hb_t = load_b(hb, E * EH, "hb")
            cbmu_t = load_b(cbmu, G, "cbmu")
            cbls2_t = load_b(cbls2, G, "cbls2")
            Ssum_t = wpool.tile([G, A], BF16, name="Ssum", tag="Ssum", bufs=1)
            nc.sync.dma_start(Ssum_t[:], Ssum[:])

            def linear_elu(n, ins, w_tiles, b_tiles, Mtot, name):
                """out.T = elu(W.T @ in.T + b) as bf16 SBUF m-tiles."""
                outs = []
                for mi, m0 in enumerate(range(0, Mtot, 128)):
                    mm = min(128, Mtot - m0)
                    ps = psum.tile(
                        [mm, NB], F32, name=f"{name}ps{mi}_{n}", tag="bb", bufs=4
                    )
                    for ki, it in enumerate(ins):
                        nc.tensor.matmul(
                            ps[:], w_tiles[ki][:, m0:m0 + mm], it[:],
                            start=(ki == 0), stop=(ki == len(ins) - 1),
                        )
                    bt = b_tiles[mi]
                    ex = apool.tile(
                        [mm, NB], BF16, name=f"{name}e{mi}_{n}",
                        tag=f"{name}e{mi}", bufs=2,
                    )
                    nc.scalar.activation(ex[:], ps[:], AF.Exp, bias=bt[:])
                    tm = apool.tile(
                        [mm, NB], BF16, name=f"{name}t{mi}_{n}",
                        tag=f"{name}t{mi}", bufs=2,
                    )
                    nc.vector.tensor_scalar(
                        tm[:], ex[:], 1.0, 1.0, OP.min, OP.subtract
                    )
                    ot = apool.tile(
                        [mm, NB], BF16, name=f"{name}o{mi}_{n}",
                        tag=f"{name}o{mi}", bufs=2,
                    )
                    nc.vector.scalar_tensor_tensor(
                        ot[:], ps[:], bt[:], tm[:], OP.add, OP.max
                    )
                    outs.append(ot)
                return outs

            for n in range(NT):
                n0 = n * NB

                def load_in(src, K, name):
                    tiles = []
                    for ki, k0 in enumerate(range(0, K, 128)):
                        kk = min(128, K - k0)
                        t = apool.tile(
                            [kk, NB], BF16, name=f"{name}{ki}_{n}",
                            tag=f"{name}{ki}", bufs=2,
                        )
                        nc.sync.dma_start(t[:], src[k0:k0 + kk, n0:n0 + NB])
                        tiles.append(t)
                    return tiles

                x_t = load_in(xT, IN_DIM, "x")
                xng_t = load_in(xngT, IN_DIM_NG, "xng")

                # Gating network.
                g1 = linear_elu(n, x_t, gW1_t, gb1_t, H1, "g1")
                g2 = linear_elu(n, g1, gW2_t, gb2_t, H2, "g2")
                w96_ps = psum.tile([G, NB], F32, name=f"w96ps_{n}", tag="bb",
                                   bufs=4)
                for ki in range(len(gW3_t)):
                    nc.tensor.matmul(
                        w96_ps[:], gW3_t[ki][:], g2[ki][:],
                        start=(ki == 0), stop=(ki == len(gW3_t) - 1),
                    )
                wb = apool.tile([G, NB], F32, name=f"wb_{n}", tag="wb", bufs=2)
                nc.scalar.activation(
                    wb[:], w96_ps[:], AF.Sigmoid, bias=gb3_t[0][:]
                )

                # Expert backbone + hidden.
                h1 = linear_elu(n, xng_t, eW1_t, eb1_t, H1, "h1")
                h2 = linear_elu(n, h1, eW2_t, eb2_t, H2, "h2")
                eh = linear_elu(n, h2, hW_t, hb_t, E * EH, "eh")

                # Grouped per-expert GEMM as block-diagonal matmuls.
                mu_ps = psum.tile([G, NB], F32, name=f"mups_{n}", tag="mu",
                                  bufs=1)
                ls_ps = psum.tile([G, NB], F32, name=f"lsps_{n}", tag="ls",
                                  bufs=1)
                for ki in range(len(Wmu_t)):
                    nc.tensor.matmul(
                        mu_ps[:], Wmu_t[ki][:], eh[ki][:],
                        start=(ki == 0), stop=(ki == len(Wmu_t) - 1),
                    )
                for ki in range(len(Wls_t)):
                    nc.tensor.matmul(
                        ls_ps[:], Wls_t[ki][:], eh[ki][:],
                        start=(ki == 0), stop=(ki == len(Wls_t) - 1),
                    )

                # Mixture epilogue.
                ivr = apool.tile([G, NB], BF16, name=f"ivr_{n}", tag="ivr",
                                 bufs=2)
                nc.scalar.activation(
                    ivr[:], ls_ps[:], AF.Exp, bias=cbls2_t[0][:], scale=-2.0
                )
                iv = apool.tile([G, NB], BF16, name=f"iv_{n}", tag="iv",
                                bufs=2)
                nc.vector.tensor_scalar(
                    iv[:], ivr[:], IVAR_LO, IVAR_HI, OP.max, OP.min
                )
                t1 = apool.tile([G, NB], BF16, name=f"t1_{n}", tag="t1",
                                bufs=2)
                nc.vector.tensor_mul(t1[:], wb[:], iv[:])
                mu_s = apool.tile([G, NB], BF16, name=f"mus_{n}", tag="mus",
                                  bufs=2)
                nc.vector.tensor_scalar(
                    mu_s[:], mu_ps[:], cbmu_t[0][:], None, OP.add
                )
                t2 = apool.tile([G, NB], BF16, name=f"t2_{n}", tag="t2",
                                bufs=2)
                nc.vector.tensor_mul(t2[:], t1[:], mu_s[:])

                ws_ps = psum.tile([A, NB], F32, name=f"wsps_{n}", tag="ws",
                                  bufs=1)
                nc.tensor.matmul(ws_ps[:], Ssum_t[:], t1[:])
                wm_ps = psum.tile([A, NB], F32, name=f"wmps_{n}", tag="wm",
                                  bufs=1)
                nc.tensor.matmul(wm_ps[:], Ssum_t[:], t2[:])

                ws1 = apool.tile([A, NB], F32, name=f"ws1_{n}", tag="ws1",
                                 bufs=2)
                nc.vector.tensor_scalar(ws1[:], ws_ps[:], 1e-9, None, OP.add)
                vt = apool.tile([A, NB], F32, name=f"vt_{n}", tag="vt",
                                bufs=2)
                nc.vector.reciprocal(vt[:], ws1[:])
                sig = apool.tile([A, NB], F32, name=f"sig_{n}", tag="sig",
                                 bufs=2)
                nc.scalar.activation(sig[:], vt[:], AF.Sqrt)
                mt = apool.tile([A, NB], F32, name=f"mt_{n}", tag="mt",
                                bufs=2)
                nc.vector.tensor_mul(mt[:], vt[:], wm_ps[:])

                nc.sync.dma_start(out[0:A, n0:n0 + NB], mt[:])
                nc.sync.dma_start(out[A:2 * A, n0:n0 + NB], sig[:])
                for ei in range(E):
                    nc.sync.dma_start(
                        out[2 * A + ei:2 * A + ei + 1, n0:n0 + NB],
                        wb[A * ei:A * ei + 1, :],
                    )

    nc.compile()
    return nc


def _prep_shared(inputs):
    bf = ml_dtypes.bfloat16
    f32 = np.float32

    def w(a):
        return np.ascontiguousarray(np.asarray(a, f32)).astype(bf)

    def b(a):
        return np.ascontiguousarray(np.asarray(a, f32).reshape(-1, 1))

    cW = np.asarray(inputs["cW"], f32)      # [E, 2A, EH]
    cb = np.asarray(inputs["cb"], f32)      # [E, 2A]
    Wmu = np.zeros((E * EH, G), f32)
    Wls = np.zeros((E * EH, G), f32)
    for e in range(E):
        Wmu[EH * e:EH * (e + 1), A * e:A * (e + 1)] = cW[e, :A, :].T
        Wls[EH * e:EH * (e + 1), A * e:A * (e + 1)] = cW[e, A:, :].T
    shared = {
        "gW1": w(inputs["gW1"]), "gb1": b(inputs["gb1"]),
        "gW2": w(inputs["gW2"]), "gb2": b(inputs["gb2"]),
        "gW3x": w(np.repeat(np.asarray(inputs["gW3"], f32), A, axis=1)),
        "gb3x": b(np.repeat(np.asarray(inputs["gb3"], f32), A)),
        "eW1": w(inputs["eW1"]), "eb1": b(inputs["eb1"]),
        "eW2": w(inputs["eW2"]), "eb2": b(inputs["eb2"]),
        "hW": w(inputs["hW"]), "hb": b(inputs["hb"]),
        "Wmu": Wmu.astype(bf), "Wls": Wls.astype(bf),
        "cbmu": b(cb[:, :A]),
        "cbls2": b(-2.0 * cb[:, A:]),
        "Ssum": np.tile(np.eye(A, dtype=f32), (E, 1)).astype(bf),
    }
    return shared


def get_nc():
    if "nc" not in _CACHE:
        _CACHE["nc"] = _build_nc()
    return _CACHE["nc"]


def make_in_maps(inputs):
    bf = ml_dtypes.bfloat16
    shared = _prep_shared(inputs)
    x = np.asarray(inputs["x"], np.float32)
    xng = np.asarray(inputs["x_no_goal"], np.float32)
    in_maps = []
    for c in range(NCORES):
        sl = slice(c * BL, (c + 1) * BL)
        m = dict(shared)
        m["xT"] = np.ascontiguousarray(x[sl].T).astype(bf)
        m["xngT"] = np.ascontiguousarray(xng[sl].T).astype(bf)
        in_maps.append(m)
    return in_maps


def unshard(results):
    full = np.concatenate(
        [np.asarray(results[c]["out"], np.float32) for c in range(NCORES)],
        axis=1,
    )  # [32, B]
    mu = np.ascontiguousarray(full[0:A].T)
    sigma = np.ascontiguousarray(full[A:2 * A].T)
    wts = np.ascontiguousarray(full[2 * A:].T)
    return mu, sigma, wts


def kernel(**inputs):
    nc = get_nc()
    in_maps = make_in_maps(inputs)
    res = bass_utils.run_bass_kernel_spmd(
        nc, in_maps, core_ids=list(range(NCORES))
    )
    return unshard(res.results)


# revision 2
# speedup vs baseline: 4.6407x; 4.6407x over previous
"""Trainium2 Bass kernel for the ActorMCP mixture-of-experts policy network.

Data-parallel over 8 NeuronCores: batch 32768 -> 4096 rows/core, weights
replicated. All activations live transposed [feature(partitions), batch(free)]
so every layer is out.T = W.T @ x.T with W in its natural [fan_in, fan_out]
layout as the stationary matmul operand.
"""

import math

import ml_dtypes
import numpy as np

import concourse.mybir as mybir
import concourse.tile as tile
from concourse import bacc, bass_utils

# Problem shape constants (fixed by the task).
B = 32768
IN_DIM = 512
IN_DIM_NG = 480
H1, H2 = 512, 256
E, EH = 8, 256
A = 12                    # ACT_DIM
G = E * A                 # 96 rows: (expert, action) pairs
NCORES = 8
BL = B // NCORES          # 4096 batch rows per core
NB = 512                  # batch columns per tile
NT = BL // NB             # batch tiles per core
OUT_ROWS = 2 * A + E      # mu(12) + sigma(12) + weights(8)

F32 = mybir.dt.float32
BF16 = mybir.dt.bfloat16
AF = mybir.ActivationFunctionType
OP = mybir.AluOpType

IVAR_LO = math.exp(-4.0)  # clip(log_std, -5, 2) in exp(-2*ls) domain
IVAR_HI = math.exp(10.0)

_CACHE: dict = {}


def _build_nc():
    nc = bacc.Bacc(
        "TRN2", target_bir_lowering=False, debug=False, num_devices=NCORES
    )

    def din(name, shape, dt):
        return nc.dram_tensor(name, list(shape), dt, kind="ExternalInput").ap()

    xT = din("xT", (IN_DIM, BL), BF16)
    xngT = din("xngT", (IN_DIM_NG, BL), BF16)
    gW1 = din("gW1", (IN_DIM, H1), BF16)
    gb1 = din("gb1", (H1, 1), F32)
    gW2 = din("gW2", (H1, H2), BF16)
    gb2 = din("gb2", (H2, 1), F32)
    gW3x = din("gW3x", (H2, G), BF16)     # gW3 columns repeated 12x
    gb3x = din("gb3x", (G, 1), F32)
    eW1 = din("eW1", (IN_DIM_NG, H1), BF16)
    eb1 = din("eb1", (H1, 1), F32)
    eW2 = din("eW2", (H1, H2), BF16)
    eb2 = din("eb2", (H2, 1), F32)
    hW = din("hW", (H2, E * EH), BF16)
    hb = din("hb", (E * EH, 1), F32)
    Wmu = din("Wmu", (E * EH, G), BF16)   # block-diagonal per-expert mu GEMM
    Wls = din("Wls", (E * EH, G), BF16)   # block-diagonal per-expert log_std
    cbmu = din("cbmu", (G, 1), F32)
    cbls2 = din("cbls2", (G, 1), F32)     # -2 * conv bias (log_std half)
    Ssum = din("Ssum", (G, A), BF16)      # sums (e, a) rows over e

    out = nc.dram_tensor("out", [OUT_ROWS, BL], F32, kind="ExternalOutput").ap()

    with tile.TileContext(nc) as tc:
        with (
            tc.tile_pool(name="wpool", bufs=1) as wpool,
            tc.tile_pool(name="apool", bufs=2) as apool,
            tc.tile_pool(name="psum", bufs=1, space="PSUM") as psum,
        ):
            def load_w(ap, K, name):
                tiles = []
                for k0 in range(0, K, 128):
                    kk = min(128, K - k0)
                    t = wpool.tile(
                        [kk, ap.shape[1]], BF16, name=f"{name}k{k0}",
                        tag=f"{name}k{k0}", bufs=1,
                    )
                    nc.sync.dma_start(t[:], ap[k0:k0 + kk, :])
                    tiles.append(t)
                return tiles

            def load_b(ap, K, name):
                tiles = []
                for k0 in range(0, K, 128):
                    kk = min(128, K - k0)
                    t = wpool.tile(
                        [kk, 1], F32, name=f"{name}k{k0}",
                        tag=f"{name}k{k0}", bufs=1,
                    )
                    nc.sync.dma_start(t[:], ap[k0:k0 + kk, :])
                    tiles.append(t)
                return tiles

            gW1_t = load_w(gW1, IN_DIM, "gW1")
            gW2_t = load_w(gW2, H1, "gW2")
            gW3_t = load_w(gW3x, H2, "gW3")
            eW1_t = load_w(eW1, IN_DIM_NG, "eW1")
            eW2_t = load_w(eW2, H1, "eW2")
            hW_t = load_w(hW, H2, "hW")
            Wmu_t = load_w(Wmu, E * EH, "Wmu")
            Wls_t = load_w(Wls, E * EH, "Wls")
            gb1_t = load_b(gb1, H1, "gb1")
            gb2_t = load_b(gb2, H2, "gb2")
            gb3_t = load_b(gb3x, G, "gb3")
            eb1_t = load_b(eb1, H1, "eb1")
            eb2_t = load_b(eb2, H2, "eb2")
            hb_t = load_b(hb, E * EH, "hb")
            cbmu_t = load_b(cbmu, G, "cbmu")
            cbls2_t = load_b(cbls2, G, "cbls2")
            Ssum_t = wpool.tile([G, A], BF16, name="Ssum", tag="Ssum", bufs=1)
            nc.sync.dma_start(Ssum_t[:], Ssum[:])

            def linear_elu(n, ins, w_tiles, b_tiles, Mtot, name):
                """out.T = elu(W.T @ in.T + b) as bf16 SBUF m-tiles."""
                outs = []
                for mi, m0 in enumerate(range(0, Mtot, 128)):
                    mm = min(128, Mtot - m0)
                    ps = psum.tile(
                        [mm, NB], F32, name=f"{name}ps{mi}_{n}", tag="bb", bufs=4
                    )
                    for ki, it in enumerate(ins):
                        nc.tensor.matmul(
                            ps[:], w_tiles[ki][:, m0:m0 + mm], it[:],
                            start=(ki == 0), stop=(ki == len(ins) - 1),
                        )
                    bt = b_tiles[mi]
                    ex = apool.tile(
                        [mm, NB], BF16, name=f"{name}e{mi}_{n}",
                        tag="elu_e", bufs=6,
                    )
                    nc.scalar.activation(ex[:], ps[:], AF.Exp, bias=bt[:])
                    tm = apool.tile(
                        [mm, NB], BF16, name=f"{name}t{mi}_{n}",
                        tag="elu_t", bufs=6,
                    )
                    nc.vector.tensor_scalar(
                        tm[:], ex[:], 1.0, 1.0, OP.min, OP.subtract
                    )
                    ot = apool.tile(
                        [mm, NB], BF16, name=f"{name}o{mi}_{n}",
                        tag=f"{name}o{mi}", bufs=2,
                    )
                    nc.vector.scalar_tensor_tensor(
                        ot[:], ps[:], bt[:], tm[:], OP.add, OP.max
                    )
                    outs.append(ot)
                return outs

            for n in range(NT):
                n0 = n * NB

                def load_in(src, K, name):
                    tiles = []
                    for ki, k0 in enumerate(range(0, K, 128)):
                        kk = min(128, K - k0)
                        t = apool.tile(
                            [kk, NB], BF16, name=f"{name}{ki}_{n}",
                            tag=f"{name}{ki}", bufs=2,
                        )
                        nc.sync.dma_start(t[:], src[k0:k0 + kk, n0:n0 + NB])
                        tiles.append(t)
                    return tiles

                x_t = load_in(xT, IN_DIM, "x")
                xng_t = load_in(xngT, IN_DIM_NG, "xng")

                # Gating network.
                g1 = linear_elu(n, x_t, gW1_t, gb1_t, H1, "g1")
                g2 = linear_elu(n, g1, gW2_t, gb2_t, H2, "g2")
                w96_ps = psum.tile([G, NB], F32, name=f"w96ps_{n}", tag="bb",
                                   bufs=4)
                for ki in range(len(gW3_t)):
                    nc.tensor.matmul(
                        w96_ps[:], gW3_t[ki][:], g2[ki][:],
                        start=(ki == 0), stop=(ki == len(gW3_t) - 1),
                    )
                wb = apool.tile([G, NB], F32, name=f"wb_{n}", tag="wb", bufs=2)
                nc.scalar.activation(
                    wb[:], w96_ps[:], AF.Sigmoid, bias=gb3_t[0][:]
                )

                # Expert backbone + hidden.
                h1 = linear_elu(n, xng_t, eW1_t, eb1_t, H1, "h1")
                h2 = linear_elu(n, h1, eW2_t, eb2_t, H2, "h2")
                eh = linear_elu(n, h2, hW_t, hb_t, E * EH, "eh")

                # Grouped per-expert GEMM as block-diagonal matmuls.
                mu_ps = psum.tile([G, NB], F32, name=f"mups_{n}", tag="mu",
                                  bufs=1)
                ls_ps = psum.tile([G, NB], F32, name=f"lsps_{n}", tag="ls",
                                  bufs=1)
                for ki in range(len(Wmu_t)):
                    nc.tensor.matmul(
                        mu_ps[:], Wmu_t[ki][:], eh[ki][:],
                        start=(ki == 0), stop=(ki == len(Wmu_t) - 1),
                    )
                for ki in range(len(Wls_t)):
                    nc.tensor.matmul(
                        ls_ps[:], Wls_t[ki][:], eh[ki][:],
                        start=(ki == 0), stop=(ki == len(Wls_t) - 1),
                    )

                # Mixture epilogue.
                ivr = apool.tile([G, NB], BF16, name=f"ivr_{n}", tag="ivr",
                                 bufs=2)
                nc.scalar.activation(
                    ivr[:], ls_ps[:], AF.Exp, bias=cbls2_t[0][:], scale=-2.0
                )
                iv = apool.tile([G, NB], BF16, name=f"iv_{n}", tag="iv",
                                bufs=2)
                nc.vector.tensor_scalar(
                    iv[:], ivr[:], IVAR_LO, IVAR_HI, OP.max, OP.min
                )
                t1 = apool.tile([G, NB], BF16, name=f"t1_{n}", tag="t1",
                                bufs=2)
                nc.vector.tensor_mul(t1[:], wb[:], iv[:])
                mu_s = apool.tile([G, NB], BF16, name=f"mus_{n}", tag="mus",
                                  bufs=2)
                nc.vector.tensor_scalar(
                    mu_s[:], mu_ps[:], cbmu_t[0][:], None, OP.add
                )
                t2 = apool.tile([G, NB], BF16, name=f"t2_{n}", tag="t2",
                                bufs=2)
                nc.vector.tensor_mul(t2[:], t1[:], mu_s[:])

                ws_ps = psum.tile([A, NB], F32, name=f"wsps_{n}", tag="ws",
                                  bufs=1)
                nc.tensor.matmul(ws_ps[:], Ssum_t[:], t1[:])
                wm_ps = psum.tile([A, NB], F32, name=f"wmps_{n}", tag="wm",
                                  bufs=1)
                nc.tensor.matmul(wm_ps[:], Ssum_t[:], t2[:])

                ws1 = apool.tile([A, NB], F32, name=f"ws1_{n}", tag="ws1",
                                 bufs=2)
                nc.vector.tensor_scalar(ws1[:], ws_ps[:], 1e-9, None, OP.add)
                vt = apool.tile([A, NB], F32, name=f"vt_{n}", tag="vt",
                                bufs=2)
                nc.vector.reciprocal(vt[:], ws1[:])
                sig = apool.tile([A, NB], F32, name=f"sig_{n}", tag="sig",
                                 bufs=2)
                nc.scalar.activation(sig[:], vt[:], AF.Sqrt)
                mt = apool.tile([A, NB], F32, name=f"mt_{n}", tag="mt",
                                bufs=2)
                nc.vector.tensor_mul(mt[:], vt[:], wm_ps[:])

                nc.sync.dma_start(out[0:A, n0:n0 + NB], mt[:])
                nc.sync.dma_start(out[A:2 * A, n0:n0 + NB], sig[:])
                for ei in range(E):
                    nc.sync.dma_start(
                        out[2 * A + ei:2 * A + ei + 1, n0:n0 + NB],
                        wb[A * ei:A * ei + 1, :],
                    )

    nc.compile()
    return nc


def _prep_shared(inputs):
    bf = ml_dtypes.bfloat16
    f32 = np.float32

    def w(a):
        return np.ascontiguousarray(np.asarray(a, f32)).astype(bf)

    def b(a):
        return np.ascontiguousarray(np.asarray(a, f32).reshape(-1, 1))

    cW = np.asarray(inputs["cW"], f32)      # [E, 2A, EH]
    cb = np.asarray(inputs["cb"], f32)      # [E, 2A]
    Wmu = np.zeros((E * EH, G), f32)
    Wls = np.zeros((E * EH, G), f32)
    for e in range(E):
        Wmu[EH * e:EH * (e + 1), A * e:A * (e + 1)] = cW[e, :A, :].T
        Wls[EH * e:EH * (e + 1), A * e:A * (e + 1)] = cW[e, A:, :].T
    shared = {
        "gW1": w(inputs["gW1"]), "gb1": b(inputs["gb1"]),
        "gW2": w(inputs["gW2"]), "gb2": b(inputs["gb2"]),
        "gW3x": w(np.repeat(np.asarray(inputs["gW3"], f32), A, axis=1)),
        "gb3x": b(np.repeat(np.asarray(inputs["gb3"], f32), A)),
        "eW1": w(inputs["eW1"]), "eb1": b(inputs["eb1"]),
        "eW2": w(inputs["eW2"]), "eb2": b(inputs["eb2"]),
        "hW": w(inputs["hW"]), "hb": b(inputs["hb"]),
        "Wmu": Wmu.astype(bf), "Wls": Wls.astype(bf),
        "cbmu": b(cb[:, :A]),
        "cbls2": b(-2.0 * cb[:, A:]),
        "Ssum": np.tile(np.eye(A, dtype=f32), (E, 1)).astype(bf),
    }
    return shared


def get_nc():
    if "nc" not in _CACHE:
        _CACHE["nc"] = _build_nc()
    return _CACHE["nc"]


def make_in_maps(inputs):
    bf = ml_dtypes.bfloat16
    shared = _prep_shared(inputs)
    x = np.asarray(inputs["x"], np.float32)
    xng = np.asarray(inputs["x_no_goal"], np.float32)
    in_maps = []
    for c in range(NCORES):
        sl = slice(c * BL, (c + 1) * BL)
        m = dict(shared)
        m["xT"] = np.ascontiguousarray(x[sl].T).astype(bf)
        m["xngT"] = np.ascontiguousarray(xng[sl].T).astype(bf)
        in_maps.append(m)
    return in_maps


def unshard(results):
    full = np.concatenate(
        [np.asarray(results[c]["out"], np.float32) for c in range(NCORES)],
        axis=1,
    )  # [32, B]
    mu = np.ascontiguousarray(full[0:A].T)
    sigma = np.ascontiguousarray(full[A:2 * A].T)
    wts = np.ascontiguousarray(full[2 * A:].T)
    return mu, sigma, wts


def kernel(**inputs):
    nc = get_nc()
    in_maps = make_in_maps(inputs)
    res = bass_utils.run_bass_kernel_spmd(
        nc, in_maps, core_ids=list(range(NCORES))
    )
    return unshard(res.results)
